# revision 25
# baseline (speedup 1.0000x reference)
"""Trainium2 Bass kernel for nn_DictMoEDirect (moe_routing), v2.

Reference computation (shapes hardcoded):
  x = hidden_states.transpose(1,0,2)              # [B,S,H]
  g = mean_s(relu(x@gW1.T + gb1) @ gW2.T + gb2)   # [B,E]
  W1_b = sum_e g[b,e] eW1[e]; b1_b = g[b]@eb1     # per-sample merged MLP
  W2_b = sum_e g[b,e] eW2[e]; b2_b = g[b]@eb2
  y = relu(x@W1_b.T + b1_b) @ W2_b.T + b2_b       # [B,S,H]
  return y.transpose(1,0,2)                       # [S,B,H]

Distribution over 8 NeuronCores (v2):
  - Gate: data-parallel (core b computes g[b] in bf16), AllGather of g.
  - Expert FFN: tensor-parallel over DFF (core j owns a 512-wide slice).
  - Weight merge (the v1 bottleneck): instead of the diag-scaled matmul
    trick (128x PE waste), use a block-diagonal gate matrix
    G16[(q,e),(q',b)] = g[b,e] * (q==q') so a single [128,128]x[128,512]
    matmul merges 16 weight rows for ALL 8 samples at once -- full use of
    the contraction dim.  Output rows are (q,b)-interleaved; a SBUF->SBUF
    DMA per (chunk, sample) deinterleaves into per-sample lhsT tiles.
  - Everything (weights, x, GEMMs, ReduceScatter partials) runs in bf16
    with fp32 PSUM accumulation; numpy-validated max-abs rel err ~6e-3
    vs the 2e-2 gate.
  - Layer-2 partial sums: 4 chunked bf16 ReduceScatters (by h-group
    pairs), overlapped with the remaining GEMM2 work.

kernel(**inputs) takes full unsharded inputs, shards/transposes/casts on
the host, runs the SPMD kernel, reassembles the full [S,B,H] output.
"""

import numpy as np

import concourse.bass as bass  # noqa: F401
import concourse.mybir as mybir
from concourse import bacc
from concourse.tile import TileContext

H = 1024
DFF = 4096
E = 8
B = 8
S = 512
NC = 8
DSL = DFF // NC  # 512, per-core DFF slice
P = 128
F32 = mybir.dt.float32
F32R = mybir.dt.float32r
BF16 = mybir.dt.bfloat16
AF = mybir.ActivationFunctionType


def build_module(debug=False, time_loop=0, time_phase=0):
    """time_loop=R wraps the FFN phases (not gate/collectives) in an
    on-device For loop for timing runs; outputs are then meaningless."""
    nc = bacc.Bacc()

    # ---- I/O ----
    xt_all = nc.declare_dram_parameter("xt_all", [B, H, S], BF16, isOutput=False)
    xt_own = nc.declare_dram_parameter("xt_own", [H, S], BF16, isOutput=False)
    gw1t = nc.declare_dram_parameter("gw1t", [H, H], BF16, isOutput=False)
    gb1t = nc.declare_dram_parameter("gb1t", [P, 8], F32, isOutput=False)
    gw2t = nc.declare_dram_parameter("gw2t", [H, E], BF16, isOutput=False)
    gb2 = nc.declare_dram_parameter("gb2", [E], F32, isOutput=False)
    # merged-expert weight streams, block-layout (see _ew1_dev/_ew2_dev)
    ew1b = nc.declare_dram_parameter("ew1b", [P, 8 * 8 * S], BF16, isOutput=False)
    ew2b = nc.declare_dram_parameter("ew2b", [P, 8 * 8 * S], BF16, isOutput=False)
    qblk = nc.declare_dram_parameter("qblk", [P, P], F32, isOutput=False)
    eb1s = nc.declare_dram_parameter("eb1s", [E, DSL], F32R, isOutput=False)
    eb2 = nc.declare_dram_parameter("eb2", [E, H], F32, isOutput=False)
    y_out = nc.declare_dram_parameter("y2t", [H, S], F32, isOutput=True)
    if debug:
        dbg_g = nc.declare_dram_parameter("dbg_g", [NC * E], F32, isOutput=True)
        dbg_w1t = nc.declare_dram_parameter("dbg_w1t", [P, 4096], BF16, isOutput=True)
        dbg_w2t = nc.declare_dram_parameter("dbg_w2t", [P, 4096], BF16, isOutput=True)
        dbg_y1 = nc.declare_dram_parameter("dbg_y1", [4, P, S], BF16, isOutput=True)

    # ---- internal DRAM ----
    ag_in = nc.dram_tensor("ag_in", [E], F32)
    ag_out = nc.dram_tensor("ag_out", [NC * E], F32, addr_space="Shared")
    ag_t = nc.dram_tensor("ag_t", [E * B], F32)
    rs_in = [nc.dram_tensor(f"rs_in{g}", [B, 2, P, S], BF16) for g in range(4)]
    rs_out = [nc.dram_tensor(f"rs_out{g}", [2 * P, S], BF16) for g in range(4)]
    groups = [list(range(NC))]

    with TileContext(nc) as tc:
        with (
            tc.tile_pool(name="main", bufs=1) as pool,
            tc.tile_pool(name="psum", bufs=2, space="PSUM") as pp,
        ):
            # ---------------- gate (own sample, bf16) ----------------
            xo = pool.tile([P, 8 * S], BF16, tag="xb", bufs=2)
            nc.sync.dma_start(
                xo[:].rearrange("p (k s) -> p k s", k=8),
                xt_own.rearrange("(k p) s -> p k s", p=P),
            )
            gb1_sb = pool.tile([P, 8], F32, tag="gb1")
            nc.sync.dma_start(gb1_sb[:], gb1t[:])
            h1 = pool.tile([P, 8 * S], BF16, tag="xb", bufs=2)
            gw1_a = pool.tile([P, 4096], BF16, tag="ew", bufs=3)
            gw1_b = pool.tile([P, 4096], BF16, tag="ew", bufs=3)
            for k in range(4):
                nc.sync.dma_start(
                    gw1_a[:, k * H : (k + 1) * H], gw1t[k * P : (k + 1) * P, :]
                )
            for k in range(4):
                nc.sync.dma_start(
                    gw1_b[:, k * H : (k + 1) * H],
                    gw1t[(4 + k) * P : (5 + k) * P, :],
                )
            for m in range(8):
                ps = pp.tile([P, S], F32, tag="out", bufs=3)
                for k in range(8):
                    gw = gw1_a if k < 4 else gw1_b
                    nc.tensor.matmul(
                        ps[:],
                        gw[:, (k % 4) * H + m * P : (k % 4) * H + (m + 1) * P],
                        xo[:, k * S : (k + 1) * S],
                        start=(k == 0),
                        stop=(k == 7),
                    )
                nc.scalar.activation(
                    h1[:, m * S : (m + 1) * S],
                    ps[:],
                    AF.Relu,
                    bias=gb1_sb[:, m : m + 1],
                )
            gw2_r = pool.tile([P, 64], BF16, tag="gw2")
            for k in range(8):
                nc.sync.dma_start(
                    gw2_r[:, k * E : (k + 1) * E], gw2t[k * P : (k + 1) * P, :]
                )
            ps_g = pp.tile([E, S], F32, tag="tiny")
            for k in range(8):
                nc.tensor.matmul(
                    ps_g[:],
                    gw2_r[:, k * E : (k + 1) * E],
                    h1[:, k * S : (k + 1) * S],
                    start=(k == 0),
                    stop=(k == 7),
                )
            gsum = pool.tile([E, 1], F32, tag="gsum")
            nc.vector.reduce_sum(gsum[:], ps_g[:], axis=mybir.AxisListType.X)
            gb2_sb = pool.tile([E, 1], F32, tag="gb2")
            nc.sync.dma_start(gb2_sb[:], gb2[:, None])
            gmean = pool.tile([E, 1], F32, tag="gmean")
            nc.vector.tensor_scalar_mul(gmean[:], gsum[:], 1.0 / S)
            gown = pool.tile([E, 1], F32, tag="gown")
            nc.vector.tensor_add(gown[:], gmean[:], gb2_sb[:])
            nc.sync.dma_start(ag_in[:], gown[:, 0])

            nc.gpsimd.collective_compute(
                "AllGather",
                mybir.AluOpType.bypass,
                ins=[ag_in[:]],
                outs=[ag_out[:]],
                replica_groups=groups,
            )
            if debug:
                nc.sync.dma_start(dbg_g[:], ag_out[:])

            # ---- block gate matrix G16[(q,e),(b,q')] = g[b,e]*(q==q')
            # (sample-major columns so merged rows for sample b land on the
            # contiguous partitions b*16..b*16+16 of the PSUM output)
            qblk_sb = pool.tile([P, P], F32, tag="qblk")
            nc.sync.dma_start(qblk_sb[:], qblk[:])
            gT_f = pool.tile([E, B], F32, tag="gTf")
            nc.gpsimd.dma_start(gT_f[:], ag_out.rearrange("(b e) -> e b", e=E))
            nc.sync.dma_start(ag_t.ap().rearrange("(e b) -> e b", e=E), gT_f[:])
            gcols = pool.tile([P, B], F32, tag="gcols")
            for q in range(16):
                nc.sync.dma_start(
                    gcols[q * 8 : (q + 1) * 8, :],
                    ag_t.ap().rearrange("(e b) -> e b", e=E),
                )
            # G16[:, b*16:(b+1)*16] = qblk[:, same] * g[b, e(p)]
            g16 = pool.tile([P, P], BF16, tag="g16")
            for b in range(B):
                nc.vector.tensor_scalar_mul(
                    g16[:, b * 16 : (b + 1) * 16],
                    qblk_sb[:, b * 16 : (b + 1) * 16],
                    gcols[:, b : b + 1],
                )

            # transposed tiny gate [E, B] for bias merge
            gT_r = pool.tile([E, B], F32R, tag="gT")
            nc.gpsimd.dma_start(gT_r[:], ag_out.rearrange("(b e) -> e b", e=E))

            # ---- merged per-sample biases ----
            # b1t[:, m*8+b] = (g[b] @ eb1s)[m-tile]        (full value)
            # b2t[:, m*8+b] = (g[b] @ eb2)[m-tile] / 8     (1/8: summed by RS)
            eb1_r = pool.tile([E, DSL], F32R, tag="eb1")
            nc.sync.dma_start(eb1_r[:], eb1s[:])
            eb2_f = pool.tile([E, H], F32, tag="eb2f")
            nc.sync.dma_start(eb2_f[:], eb2[:])
            eb2_r8 = pool.tile([E, H], F32R, tag="eb2r")
            nc.scalar.activation(eb2_r8[:], eb2_f[:], AF.Copy, scale=1.0 / NC)
            b1t = pool.tile([P, 4 * B], F32, tag="b1t")
            b2t = pool.tile([P, 8 * B], F32, tag="b2t")
            for mt in range(4):
                ps = pp.tile([P, B], F32, tag="tiny")
                nc.tensor.matmul(
                    ps[:],
                    eb1_r[:, mt * P : (mt + 1) * P],
                    gT_r[:],
                    start=True,
                    stop=True,
                )
                nc.vector.tensor_copy(b1t[:, mt * B : (mt + 1) * B], ps[:])
            for m in range(8):
                ps = pp.tile([P, B], F32, tag="tiny")
                nc.tensor.matmul(
                    ps[:],
                    eb2_r8[:, m * P : (m + 1) * P],
                    gT_r[:],
                    start=True,
                    stop=True,
                )
                nc.vector.tensor_copy(b2t[:, m * B : (m + 1) * B], ps[:])

            def ffn_body(with_dbg=False):
                """Merge + GEMM phases (no collectives)."""
                # per-sample merged weight tiles, [128, 4096] bf16 each.
                # w1t_b: [h_loc=(c*16+q), k*512 + o]   (lhsT for GEMM1)
                # w2t_b: [d_loc=(c*16+q), kt*1024 + h] (lhsT for GEMM2)
                w1t = [
                    pool.tile([P, 4096], BF16, tag="wmt", bufs=11, name=f"w1t{b}")
                    for b in range(B)
                ]
                w2t = [
                    pool.tile([P, 4096], BF16, tag="wmt", bufs=11, name=f"w2t{b}")
                    for b in range(B)
                ]
                y1 = [
                    [
                        pool.tile([P, S], BF16, tag="y1", bufs=32, name=f"y1_{b}_{m}")
                        for m in range(4)
                    ]
                    for b in range(B)
                ]

                # ---- merge layer 1: 64 matmuls, all samples at once ----
                # merged rows for sample b land on contiguous partitions
                # b*16..b*16+16 of the chunk; the deinterleave DMA (one per
                # chunk and sample, contiguous partition slices both sides)
                # moves them to partitions h_loc = c*16+q of w1t[b].
                for k in range(8):
                    ewt = pool.tile([P, 4096], BF16, tag="ew", bufs=3)
                    nc.sync.dma_start(ewt[:], ew1b[:, k * 4096 : (k + 1) * 4096])
                    for c in range(8):
                        ps = pp.tile([P, S], F32, tag="mm", bufs=3)
                        nc.tensor.matmul(
                            ps[:],
                            g16[:],
                            ewt[:, c * S : (c + 1) * S],
                            start=True,
                            stop=True,
                        )
                        stg = pool.tile([P, S], BF16, tag="stg", bufs=6)
                        if c % 2 == 0:
                            nc.vector.tensor_copy(stg[:], ps[:])
                        else:
                            nc.scalar.activation(stg[:], ps[:], AF.Copy)
                        for b in range(B):
                            eng = nc.sync if b < 4 else nc.scalar
                            eng.dma_start(
                                w1t[b][
                                    c * 16 : (c + 1) * 16, k * S : (k + 1) * S
                                ],
                                stg[b * 16 : (b + 1) * 16, :],
                            )

                # ---- merge layer 2 (deferred until after GEMM1 so the
                # wmt buffer reuse w1t->w2t only depends on already-issued
                # gemm1 completions -- avoids DMA-queue deadlock) ----
                def merge2_group(grp):
                    kt, half = grp // 2, grp % 2
                    ewt = pool.tile([P, 4096], BF16, tag="ew", bufs=3)
                    nc.sync.dma_start(
                        ewt[:], ew2b[:, grp * 4096 : (grp + 1) * 4096]
                    )
                    off = kt * 1024 + half * S
                    for c in range(8):
                        ps = pp.tile([P, S], F32, tag="mm", bufs=3)
                        nc.tensor.matmul(
                            ps[:],
                            g16[:],
                            ewt[:, c * S : (c + 1) * S],
                            start=True,
                            stop=True,
                        )
                        stg = pool.tile([P, S], BF16, tag="stg", bufs=6)
                        if c % 2 == 0:
                            nc.vector.tensor_copy(stg[:], ps[:])
                        else:
                            nc.scalar.activation(stg[:], ps[:], AF.Copy)
                        for b in range(B):
                            eng = nc.sync if b < 4 else nc.scalar
                            eng.dma_start(
                                w2t[b][c * 16 : (c + 1) * 16, off : off + S],
                                stg[b * 16 : (b + 1) * 16, :],
                            )

                for b in range(B):
                    xb = pool.tile([P, 8 * S], BF16, tag="xb", bufs=2)
                    nc.sync.dma_start(
                        xb[:].rearrange("p (k s) -> p k s", k=8),
                        xt_all.rearrange("b (k p) s -> b p k s", p=P)[b],
                    )
                    for m in range(4):
                        ps = pp.tile([P, S], F32, tag="out", bufs=3)
                        for k in range(8):
                            nc.tensor.matmul(
                                ps[:],
                                w1t[b][:, k * S + m * P : k * S + (m + 1) * P],
                                xb[:, k * S : (k + 1) * S],
                                start=(k == 0),
                                stop=(k == 7),
                            )
                        nc.scalar.activation(
                            y1[b][m][:],
                            ps[:],
                            AF.Relu,
                            bias=b1t[:, m * B + b : m * B + b + 1],
                        )

                for grp in range(8):
                    merge2_group(grp)

                if with_dbg:
                    nc.sync.dma_start(dbg_w1t[:], w1t[0][:])
                    nc.sync.dma_start(dbg_w2t[:], w2t[0][:])
                    for m in range(4):
                        nc.sync.dma_start(dbg_y1[m], y1[0][m][:])

                # ---- GEMM2, h-group-major so RS can be chunked ----
                for gr in range(4):
                    for b in range(B):
                        for mi in range(2):
                            m = gr * 2 + mi
                            ps = pp.tile([P, S], F32, tag="out", bufs=3)
                            for kt in range(4):
                                nc.tensor.matmul(
                                    ps[:],
                                    w2t[b][
                                        :,
                                        kt * 1024 + m * P : kt * 1024 + (m + 1) * P,
                                    ],
                                    y1[b][kt][:],
                                    start=(kt == 0),
                                    stop=(kt == 3),
                                )
                            y2 = pool.tile([P, S], BF16, tag="y2", bufs=4)
                            nc.scalar.activation(
                                y2[:],
                                ps[:],
                                AF.Identity,
                                bias=b2t[:, m * B + b : m * B + b + 1],
                            )
                            nc.sync.dma_start(rs_in[gr][b, mi], y2[:])

            if time_loop:
                with tc.For_i(0, time_loop, 1):
                    ffn_body()
                nc.gpsimd.dma_start(
                    y_out[0 : 2 * P],
                    rs_in[0].ap()[0].rearrange("m p s -> (m p) s"),
                )
            else:
                ffn_body(with_dbg=debug)
                for gr in range(4):
                    nc.gpsimd.collective_compute(
                        "ReduceScatter",
                        mybir.AluOpType.add,
                        ins=[rs_in[gr].ap().rearrange("b m p s -> (b m p) s")],
                        outs=[rs_out[gr][:]],
                        replica_groups=groups,
                    )
                for gr in range(4):
                    # cast bf16 -> fp32 on the way out (SWDGE cast DMA)
                    nc.gpsimd.dma_start(
                        y_out[gr * 256 : (gr + 1) * 256], rs_out[gr][:]
                    )

    nc.compile()
    return nc


def _to_bf16(a):
    import ml_dtypes

    return np.asarray(a, np.float32).astype(ml_dtypes.bfloat16)


def _ew1_dev(a):
    # a: [E, DSL(o), H(h)] -> [128=(q*8+e), (k 8, c 8, o 512)]
    # h = k*128 + c*16 + q
    a2 = np.asarray(a, np.float32).reshape(E, DSL, 8, 8, 16)
    a3 = a2.transpose(4, 0, 2, 3, 1)  # [q, e, k, c, o]
    return _to_bf16(np.ascontiguousarray(a3.reshape(P, 8 * 8 * S)))


def _ew2_dev(c):
    # c: [E, H(hh), DSL(d)] -> [128=(q*8+e), (kt 4, half 2, c 8, h' 512)]
    # d = kt*128 + c*16 + q ; hh = half*512 + h'
    c2 = np.asarray(c, np.float32).reshape(E, 2, S, 4, 8, 16)
    c3 = c2.transpose(5, 0, 3, 1, 4, 2)  # [q, e, kt, half, c, h']
    return _to_bf16(np.ascontiguousarray(c3.reshape(P, 8 * 8 * S)))


def _shard_inputs(hidden_states, gW1, gb1, gW2, gb2, eW1, eb1, eW2, eb2):
    xt_all = _to_bf16(
        np.ascontiguousarray(np.asarray(hidden_states, np.float32).transpose(1, 2, 0))
    )  # [B, H, S] bf16
    gW1t = _to_bf16(np.ascontiguousarray(np.asarray(gW1, np.float32).T))
    gb1t = np.ascontiguousarray(np.asarray(gb1, np.float32).reshape(8, P).T)
    gW2t = _to_bf16(np.ascontiguousarray(np.asarray(gW2, np.float32).T))
    gb2 = np.ascontiguousarray(np.asarray(gb2, np.float32))
    eW1 = np.asarray(eW1, np.float32)
    eW2 = np.asarray(eW2, np.float32)
    eb1 = np.asarray(eb1, np.float32)
    eb2 = np.ascontiguousarray(np.asarray(eb2, np.float32))
    # qblk[p, m] = 1 where q(p) == q'(m): p = (q,e) q-major, m = (b,q') b-major
    qblk = (
        np.arange(P)[:, None] // 8 == np.arange(P)[None, :] % 16
    ).astype(np.float32)
    in_maps = []
    for j in range(NC):
        sl = slice(j * DSL, (j + 1) * DSL)
        in_maps.append(
            {
                "xt_all": xt_all,
                "xt_own": np.ascontiguousarray(xt_all[j]),
                "gw1t": gW1t,
                "gb1t": gb1t,
                "gw2t": gW2t,
                "gb2": gb2,
                "ew1b": _ew1_dev(eW1[:, sl, :]),
                "ew2b": _ew2_dev(eW2[:, :, sl]),
                "qblk": qblk,
                "eb1s": np.ascontiguousarray(eb1[:, sl]),
                "eb2": eb2,
            }
        )
    return in_maps


# ---------------- SPMD runner (persistent jit over axon PJRT) -----------

_CACHE = {}


def _build_runner(debug=False, time_loop=0, time_phase=0):
    import jax
    from jax.sharding import Mesh, PartitionSpec
    from jax.experimental.shard_map import shard_map
    from concourse import bass2jax

    nc = build_module(debug=debug, time_loop=time_loop, time_phase=time_phase)
    bass2jax.install_neuronx_cc_hook()
    partition_name = nc.partition_id_tensor.name if nc.partition_id_tensor else None

    in_names, out_names, out_avals = [], [], []
    for alloc in nc.m.functions[0].allocations:
        if not isinstance(alloc, mybir.MemoryLocationSet):
            continue
        name = alloc.memorylocations[0].name
        if alloc.kind == "ExternalInput":
            if name != partition_name:
                in_names.append(name)
        elif alloc.kind == "ExternalOutput":
            out_avals.append(
                jax.core.ShapedArray(
                    tuple(alloc.tensor_shape), mybir.dt.np(alloc.dtype)
                )
            )
            out_names.append(name)
    n_outs = len(out_names)
    all_in_names = list(in_names) + list(out_names)
    if partition_name is not None:
        all_in_names.append(partition_name)

    def _body(*args):
        operands = list(args)
        if partition_name is not None:
            operands.append(bass2jax.partition_id_tensor())
        return tuple(
            bass2jax._bass_exec_p.bind(
                *operands,
                out_avals=tuple(out_avals),
                in_names=tuple(all_in_names),
                out_names=tuple(out_names),
                lowering_input_output_aliases=(),
                sim_require_finite=True,
                sim_require_nnan=True,
                nc=nc,
            )
        )

    devices = jax.devices()[:NC]
    mesh = Mesh(np.asarray(devices), ("core",))
    n_params = len(in_names)
    sharded = jax.jit(
        shard_map(
            _body,
            mesh=mesh,
            in_specs=(PartitionSpec("core"),) * (n_params + n_outs),
            out_specs=(PartitionSpec("core"),) * n_outs,
            check_rep=False,
        ),
        keep_unused=True,
    )
    zero_shapes = [((NC * a.shape[0], *a.shape[1:]), a.dtype) for a in out_avals]

    def run(in_maps, device_inputs=None, fetch=True):
        if device_inputs is None:
            concat_in = [
                np.concatenate(
                    [np.asarray(in_maps[c][n]) for c in range(NC)], axis=0
                )
                for n in in_names
            ]
            dev_params = [jax.device_put(x) for x in concat_in]
            dev_zeros = [jax.device_put(np.zeros(s, d)) for s, d in zero_shapes]
            device_inputs = (dev_params, dev_zeros)
            jax.block_until_ready(dev_params)
            jax.block_until_ready(dev_zeros)
        dev_params, dev_zeros = device_inputs
        out_arrs = sharded(*dev_params, *dev_zeros)
        jax.block_until_ready(out_arrs)
        if not fetch:
            return None, device_inputs
        results = [
            {
                name: np.asarray(out_arrs[i]).reshape(NC, *out_avals[i].shape)[c]
                for i, name in enumerate(out_names)
            }
            for c in range(NC)
        ]
        return results, device_inputs

    return run


def get_runner(debug=False, time_loop=0, time_phase=0):
    key = ("run", debug, time_loop, time_phase)
    if key not in _CACHE:
        _CACHE[key] = _build_runner(
            debug=debug, time_loop=time_loop, time_phase=time_phase
        )
    return _CACHE[key]


def kernel(**inputs) -> np.ndarray:
    run = get_runner()
    in_maps = _shard_inputs(**inputs)
    results, _ = run(in_maps)
    # core b's output is y2^T[b] = [H, S]; assemble [S, B, H]
    y2t = np.stack([results[b]["y2t"] for b in range(B)], axis=0)  # [B, H, S]
    return np.ascontiguousarray(y2t.transpose(2, 0, 1)).astype(np.float32)
